# revision 1
# baseline (speedup 1.0000x reference)
"""CrossWindowAttention Trainium2 kernel (8 NeuronCores, data-parallel over B).

v2 redesign vs baseline (785us):
  - bias folds (454us of PE identity matmuls) eliminated: host precomputes
    eb = exp(maskT + rpbT) bf16; kernel applies E = exp(A) * eb as a DVE/Pool
    elementwise multiply (exp(A+b) == exp(A)*exp(b)).
  - r-broadcast outer-product matmuls write into the pv PSUM banks after
    evacuation (no extra banks); r's ln/exp shrunk to [6,344] ACT ops.
  - all DMAs moved off the Scalar engine (ACT does only exp/ln); issue from
    Sync (hwdge) and GpSimd (swdge); elementwise spread DVE/Pool.
  - out projection in bf16 (ap=192 at 1 cyc/row instead of fp32r's 4).
  - 2-head QK groups, a-slots double-buffered: PSUM = 2x2 (a) + pv0 + pv1
    + s = 7 banks.
"""

import sys

if "/opt/trn_rl_repo" not in sys.path:
    sys.path.insert(0, "/opt/trn_rl_repo")

import numpy as np
import ml_dtypes
from contextlib import ExitStack

import concourse.bass as bass
import concourse.tile as tile
from concourse import bacc, mybir
from concourse.bass_utils import run_bass_kernel_spmd

dt = mybir.dt

DEBUG = False

B = 256
N = 343          # tokens per window
NP = 344         # padded token dim
C = 192
H = 6
D = 32
NW = 64
NCORES = 8
BL = B // NCORES
CA = 194         # c + ones row + zero pad row
SCALE = D ** -0.5

BF16 = dt.bfloat16
F32 = dt.float32

KCH = [(0, 128), (128, 66)]              # contraction chunks of augmented c
MCH = [(0, 96), (96, 96)]                # q/k head-tile chunks
TCH = [(0, 128), (128, 128), (256, 88)]  # token chunks (row-padded to 344)
NG = 3                                   # QK head groups (2 heads each)


def _patch_act_tables():
    """Force one ACT table set covering both Exp and Ln (avoids per-batch
    ACT_TABLE_LOAD ~9.5us)."""
    import concourse.bacc as bacc_mod
    if getattr(bacc_mod, "_act_tables_patched", False):
        return
    real = bacc_mod.get_activation_tables

    def patched(arch):
        t = real(arch)
        return {k: (v if k == "natural_log_exp_and_others" else set())
                for k, v in t.items()}

    bacc_mod.get_activation_tables = patched
    bacc_mod._act_tables_patched = True


def build_program():
    _patch_act_tables()
    nc = bacc.Bacc("TRN2", target_bir_lowering=False, debug=False,
                   num_devices=NCORES)

    d_x = nc.dram_tensor("x", [BL, 2, CA, NP], BF16, kind="ExternalInput").ap()
    d_eb = nc.dram_tensor("eb", [BL, NP, H * NP], BF16, kind="ExternalInput").ap()
    d_wq = nc.dram_tensor("wq", [CA, C], BF16, kind="ExternalInput").ap()
    d_wk = nc.dram_tensor("wk", [CA, C], BF16, kind="ExternalInput").ap()
    d_wv = nc.dram_tensor("wv", [CA, C], BF16, kind="ExternalInput").ap()
    d_wp = nc.dram_tensor("wp", [194, C], BF16, kind="ExternalInput").ap()
    d_ones = nc.dram_tensor("onesr", [1, NP], BF16, kind="ExternalInput").ap()
    d_onesb = nc.dram_tensor("onesb", [1, 32], BF16, kind="ExternalInput").ap()
    d_onesc = nc.dram_tensor("onesc", [128, 2], BF16, kind="ExternalInput").ap()
    d_out = nc.dram_tensor("out", [BL, N, C], F32, kind="ExternalOutput").ap()
    dbg = {}
    if DEBUG:
        for nm, shp, dty in [("dbg_qt", [96, NP], BF16), ("dbg_kt", [96, NP], BF16),
                             ("dbg_v", [128, C], BF16), ("dbg_e", [128, 2 * NP], BF16),
                             ("dbg_e2", [128, 2 * NP], BF16), ("dbg_rs", [6, NP], F32),
                             ("dbg_rsb", [6, NP], BF16),
                             ("dbg_ost0", [128, NP], BF16), ("dbg_ost1", [65, NP], BF16)]:
            dbg[nm] = nc.dram_tensor(nm, shp, dty, kind="ExternalOutput").ap()

    with tile.TileContext(nc) as tc, ExitStack() as ctx:
        cpool = ctx.enter_context(tc.tile_pool(name="const", bufs=1))
        xpool = ctx.enter_context(tc.tile_pool(name="x", bufs=2))
        ebpool = ctx.enter_context(tc.tile_pool(name="ebp", bufs=2))
        qkpool = ctx.enter_context(tc.tile_pool(name="qk", bufs=2))
        vpool = ctx.enter_context(tc.tile_pool(name="v", bufs=2))
        epool = ctx.enter_context(tc.tile_pool(name="e", bufs=3))
        e2pool = ctx.enter_context(tc.tile_pool(name="e2", bufs=3))
        evpool = ctx.enter_context(tc.tile_pool(name="ev", bufs=2))
        rpool = ctx.enter_context(tc.tile_pool(name="r", bufs=2))
        opool = ctx.enter_context(tc.tile_pool(name="o", bufs=2))
        fpool = ctx.enter_context(tc.tile_pool(name="fin", bufs=2))
        ps_a = ctx.enter_context(tc.tile_pool(name="ps_a", bufs=2, space="PSUM"))
        ps_p0 = ctx.enter_context(tc.tile_pool(name="ps_p0", bufs=1, space="PSUM"))
        ps_p1 = ctx.enter_context(tc.tile_pool(name="ps_p1", bufs=1, space="PSUM"))
        ps_s = ctx.enter_context(tc.tile_pool(name="ps_s", bufs=1, space="PSUM"))
        ps_rb = ctx.enter_context(tc.tile_pool(name="ps_rb", bufs=1, space="PSUM"))

        # ---- resident constants ----
        wq_t, wk_t, wv_t = [], [], []
        for ki, (ko, kn) in enumerate(KCH):
            for lst, src, nm in ((wq_t, d_wq, "wq"), (wk_t, d_wk, "wk"),
                                 (wv_t, d_wv, "wv")):
                t = cpool.tile([kn, C], BF16, tag=f"{nm}{ki}", name=f"{nm}{ki}")
                nc.sync.dma_start(t[:], src[ko:ko + kn, :])
                lst.append(t)
        wp0 = cpool.tile([128, C], BF16, tag="wp0")
        nc.sync.dma_start(wp0[:], d_wp[0:128, :])
        wp1 = cpool.tile([65, C], BF16, tag="wp1")
        nc.sync.dma_start(wp1[:], d_wp[128:193, :])
        onesb_t = cpool.tile([1, 32], BF16, tag="onesb")
        nc.sync.dma_start(onesb_t[:], d_onesb[:, :])
        onesc_t = cpool.tile([128, 2], BF16, tag="onesc")
        nc.sync.dma_start(onesc_t[:], d_onesc[:, :])

        def head_tile(h):
            # head h lives in q/k tile h//3 at row 32*(h%3)
            return h // 3, 32 * (h % 3)

        def emit_head(b, tail_fn=None):
            # ---- input DMAs (Sync hwdge) ----
            x_t = []
            for ki, (ko, kn) in enumerate(KCH):
                t = xpool.tile([kn, 2, NP], BF16, tag=f"x{ki}", name=f"x{ki}")
                nc.sync.dma_start(t[:], d_x[b, :, ko:ko + kn, :].rearrange("a k n -> k a n"))
                x_t.append(t)
            eb_t = []
            for ci, (to, tn) in enumerate(TCH):
                t = ebpool.tile([tn, H * NP], BF16, tag=f"eb{ci}", name=f"eb{ci}")
                nc.sync.dma_start(t[:], d_eb[b, to:to + tn, :])
                eb_t.append(t)

            # ---- previous batch's tail (r chain + out proj) ----
            if tail_fn is not None:
                tail_fn()

            # ---- Q^T / K^T projections ----
            qt, kt = [], []
            for w_t, xi, dest, nm in ((wq_t, 0, None, "q"), (wk_t, 1, None, "k")):
                dest = qt if nm == "q" else kt
                for mi, (mo, mn) in enumerate(MCH):
                    ps = ps_a.tile([128, 2, 512], F32, tag="a", name="mmq")
                    for ki, (ko, kn) in enumerate(KCH):
                        nc.tensor.matmul(ps[0:mn, 0, 0:NP],
                                         w_t[ki][:, mo:mo + mn],
                                         x_t[ki][:, xi, :],
                                         start=(ki == 0), stop=(ki == len(KCH) - 1))
                    sb = qkpool.tile([96, NP], BF16, tag=f"{nm}{mi}", name=f"{nm}{mi}")
                    nc.vector.tensor_copy(sb[:], ps[0:mn, 0, 0:NP])
                    if DEBUG and b == 0 and mi == 0:
                        nc.sync.dma_start(dbg[f"dbg_{nm}t"][:, :], sb[:])
                    dest.append(sb)

            # ---- V projection ----
            v_t = []
            for ti, (to, tn) in enumerate(TCH):
                ps = ps_a.tile([128, 2, 512], F32, tag="a", name="mmv")
                for ki, (ko, kn) in enumerate(KCH):
                    nc.tensor.matmul(ps[0:tn, 0, 0:C],
                                     x_t[ki][:, 1, to:to + tn],
                                     wv_t[ki][:],
                                     start=(ki == 0), stop=(ki == len(KCH) - 1))
                sb = vpool.tile([128, C], BF16, tag=f"v{ti}", name=f"v{ti}")
                nc.vector.tensor_copy(sb[0:tn, :], ps[0:tn, 0, 0:C])
                if DEBUG and b == 0 and ti == 0:
                    nc.sync.dma_start(dbg["dbg_v"][:, :], sb[:])
                v_t.append(sb)

            # ---- attention ----
            pv0 = ps_p0.tile([128, 512], F32, tag="pv0", name="pv0")
            pv1 = ps_p1.tile([128, 512], F32, tag="pv1", name="pv1")
            s_ps = ps_s.tile([128, 512], F32, tag="s", name="s")
            for g in range(NG):
                for ci, (co_, cn) in enumerate(TCH):
                    a_ps = ps_a.tile([128, 2, 512], F32, tag="a", name="aqk")
                    for hh in range(2):
                        h = 2 * g + hh
                        t_i, r_off = head_tile(h)
                        nc.tensor.matmul(
                            a_ps[0:cn, hh, 0:NP],
                            kt[t_i][r_off:r_off + D, co_:co_ + cn],
                            qt[t_i][r_off:r_off + D, :],
                            start=True, stop=True)
                    e_t = epool.tile([128, 2 * NP], BF16, tag="e", name="e")
                    nc.scalar.activation(
                        e_t[0:cn, :].rearrange("p (r n) -> p r n", r=2),
                        a_ps[0:cn, 0:2, 0:NP],
                        mybir.ActivationFunctionType.Exp)
                    e2 = e2pool.tile([128, 2 * NP], BF16, tag="e2", name="e2")
                    nc.vector.tensor_mul(
                        e2[0:cn, :], e_t[0:cn, :],
                        eb_t[ci][0:cn, 2 * g * NP:(2 * g + 2) * NP])
                    if DEBUG and b == 0 and g == 0 and ci == 0:
                        nc.sync.dma_start(dbg["dbg_e"][:, :], e_t[:, :])
                        nc.sync.dma_start(dbg["dbg_e2"][:, :], e2[:, :])
                    for hh in range(2):
                        h = 2 * g + hh
                        bank, base = (pv0, 32 * h) if h < 4 else (pv1, 32 * (h - 4))
                        nc.tensor.matmul(
                            bank[base:base + D, 0:NP],
                            v_t[ci][0:cn, 32 * h:32 * h + D],
                            e2[0:cn, hh * NP:(hh + 1) * NP],
                            start=(ci == 0), stop=(ci == len(TCH) - 1),
                            tile_position=(0, base), skip_group_check=True)
                        if h < 4:
                            s_out = s_ps[32 * h:32 * h + 1, 0:NP]
                            s_tp = (0, 32 * h)
                        else:
                            s_out = pv1[32 * (h - 2):32 * (h - 2) + 1, 0:NP]
                            s_tp = (0, 32 * (h - 2))
                        nc.tensor.matmul(
                            s_out,
                            onesc_t[0:cn, 0:1],
                            e2[0:cn, hh * NP:(hh + 1) * NP],
                            start=(ci == 0), stop=(ci == len(TCH) - 1),
                            tile_position=s_tp, skip_group_check=True)
            return pv0, pv1, s_ps

        def emit_evac(b, handles):
            pv0, pv1, s_ps = handles
            # s rows live at partitions {0,32,64,96} of s_ps (h0-3) and
            # {64,96} of pv1 (h4,5); copy to SBUF so DMA can gather them.
            ev0 = evpool.tile([128, NP], F32, tag="ev0", name="ev0")
            ev1 = evpool.tile([128, NP], F32, tag="ev1", name="ev1")
            evs = evpool.tile([97, NP], F32, tag="evs", name="evs")
            nc.vector.tensor_copy(ev0[:], pv0[0:128, 0:NP])
            nc.vector.tensor_copy(ev1[:], pv1[0:128, 0:NP])
            nc.vector.tensor_copy(evs[:], s_ps[0:97, 0:NP])
            rs = rpool.tile([6, NP], F32, tag="rs", name="rs")
            rowlen = evs[:].tensor.shape[-1]
            src = bass.AP(tensor=evs[:].tensor, offset=evs[:].offset,
                          ap=[[32 * rowlen, 4], [1, NP]])
            nc.gpsimd.dma_start(rs[0:4, :], src)
            rowlen1 = ev1[:].tensor.shape[-1]
            src = bass.AP(tensor=ev1[:].tensor,
                          offset=ev1[:].offset + 64 * rowlen1,
                          ap=[[32 * rowlen1, 2], [1, NP]])
            nc.gpsimd.dma_start(rs[4:6, :], src)
            if DEBUG and b == 0:
                nc.sync.dma_start(dbg["dbg_rs"][:, :], rs[:])
            return ev0, ev1, rs

        def emit_tail(b, handles):
            ev0, ev1, rs = handles
            # r = exp(-ln(s)) on ACT, [6, NP]
            lnr = rpool.tile([6, NP], F32, tag="lnr", name="lnr")
            nc.scalar.activation(lnr[:], rs[:],
                                 mybir.ActivationFunctionType.Ln)
            r_sb = rpool.tile([6, NP], BF16, tag="rsb", name="rsb")
            nc.scalar.activation(r_sb[:], lnr[:],
                                 mybir.ActivationFunctionType.Exp, scale=-1.0)
            # gather 6 r rows into one partition, then broadcast via K=1
            # outer-product matmuls (PE) into spare/s PSUM banks
            rmv = rpool.tile([1, 6 * NP], BF16, tag="rmv", name="rmv")
            rowlen = r_sb[:].tensor.shape[-1]
            src = bass.AP(tensor=r_sb[:].tensor, offset=r_sb[:].offset,
                          ap=[[rowlen, 6], [1, NP]])
            nc.gpsimd.dma_start(rmv[0:1, :], src)
            rb0 = ps_rb.tile([128, 512], F32, tag="rb", name="rb0")
            rb1 = ps_s.tile([128, 512], F32, tag="s", name="rb1")
            for h in range(4):
                nc.tensor.matmul(rb0[32 * h:32 * h + 32, 0:NP],
                                 onesb_t[0:1, :],
                                 rmv[0:1, h * NP:(h + 1) * NP],
                                 start=True, stop=True, tile_position=(0, 32 * h),
                                 skip_group_check=True)
            for h in range(2):
                nc.tensor.matmul(rb1[32 * h:32 * h + 32, 0:NP],
                                 onesb_t[0:1, :],
                                 rmv[0:1, (4 + h) * NP:(5 + h) * NP],
                                 start=True, stop=True, tile_position=(0, 32 * h),
                                 skip_group_check=True)
            # normalize: ost = pv * rb  (bf16 out, PSUM read on DVE)
            ost0 = opool.tile([128, NP], BF16, tag="ost0", name="ost0")
            ost1 = opool.tile([65, NP], BF16, tag="ost1", name="ost1")
            nc.vector.scalar_tensor_tensor(
                ost0[:], ev0[:], 1.0, rb0[0:128, 0:NP],
                mybir.AluOpType.mult, mybir.AluOpType.mult)
            nc.vector.scalar_tensor_tensor(
                ost1[0:64, :], ev1[0:64, :], 1.0, rb1[0:64, 0:NP],
                mybir.AluOpType.mult, mybir.AluOpType.mult)
            if DEBUG and b == 0:
                nc.sync.dma_start(dbg["dbg_rsb"][:, :], r_sb[:])
                nc.sync.dma_start(dbg["dbg_ost0"][:, :], ost0[:])
                nc.sync.dma_start(dbg["dbg_ost1"][:, :], ost1[:])
            if b < 2:  # bufs=2: the ones row persists per slot
                nc.sync.dma_start(ost1[64:65, :], d_ones[:, :])
            # output projection (bf16) + store
            for ti, (to, tn) in enumerate(TCH):
                ps = ps_a.tile([128, 2, 512], F32, tag="a", name="mmo")
                nc.tensor.matmul(ps[0:tn, 0, 0:C], ost0[:, to:to + tn], wp0[:],
                                 start=True, stop=False)
                nc.tensor.matmul(ps[0:tn, 0, 0:C], ost1[:, to:to + tn], wp1[:],
                                 start=False, stop=True)
                f_sb = fpool.tile([128, C], F32, tag=f"f{ti}", name=f"f{ti}")
                nc.scalar.copy(f_sb[0:tn, :], ps[0:tn, 0, 0:C])
                rows = min(tn, N - to)
                nc.sync.dma_start(d_out[b, to:to + rows, :], f_sb[0:rows, :])

        # software pipeline: batch b's tail is injected between batch b+1's
        # projections and attention.
        prev = None
        for b in range(BL):
            if prev is not None:
                pb, ph = prev
                tail_fn = lambda pb=pb, ph=ph: emit_tail(pb, ph)
            else:
                tail_fn = None
            ps_handles = emit_head(b, tail_fn)
            prev = (b, emit_evac(b, ps_handles))
        emit_tail(prev[0], prev[1])

    nc.compile()
    return nc


_NC_CACHE = None


def _get_program():
    global _NC_CACHE
    if _NC_CACHE is None:
        _NC_CACHE = build_program()
    return _NC_CACHE


def _prep_inputs(x_q, x_kv, mask, q_w, q_b, kv_w, kv_b, proj_w, proj_b,
                 rpb_table, rpi):
    bf16 = ml_dtypes.bfloat16
    f32 = np.float32

    def aug_w(w, bias, scale=1.0):
        m = np.zeros((CA, C), f32)
        m[:C] = np.asarray(w, f32).T
        m[C] = np.asarray(bias, f32)
        return np.ascontiguousarray(m * scale)

    wq = aug_w(q_w, q_b, SCALE).astype(bf16)
    wk = aug_w(kv_w[:C], kv_b[:C]).astype(bf16)
    wv = aug_w(kv_w[C:], kv_b[C:]).astype(bf16)
    wp = np.zeros((194, C), f32)
    wp[:C] = np.asarray(proj_w, f32).T
    wp[C] = np.asarray(proj_b, f32)
    # out-proj chunks: rows 0:128 and 128:193 (64 dims + bias at row 192)
    wp = np.concatenate([wp[0:128], wp[128:193], np.zeros((1, C), f32)], 0)
    wp = wp.astype(bf16)

    def xT_aug(x):
        out = np.zeros((B, CA, NP), f32)
        out[:, :C, :N] = np.asarray(x, f32).transpose(0, 2, 1)
        out[:, C, :N] = 1.0
        return out

    xs = np.stack([xT_aug(x_q), xT_aug(x_kv)], 1).astype(bf16)  # [B,2,CA,NP]

    # eb = exp(maskT + rpbT), bf16, per window: [NW, NP, H*NP]
    g = np.asarray(rpb_table, f32)[np.asarray(rpi)]        # [q, k, H]
    rpbT = np.zeros((NP, H, NP), f32)
    rpbT[:N, :, :N] = g.transpose(1, 2, 0)                 # [k, h, q]
    maskT = np.full((NW, NP, NP), -100.0, f32)
    maskT[:, :N, :N] = np.asarray(mask, f32).transpose(0, 2, 1)
    eb = np.empty((NW, NP, H * NP), bf16)
    for w in range(NW):
        t = np.exp(maskT[w][:, None, :] + rpbT)            # [k, h, q]
        t[0, :, N] = 1.0                                   # pad-query col: s=1
        eb[w] = t.reshape(NP, H * NP).astype(bf16)

    onesr = np.ones((1, NP), f32).astype(bf16)
    onesb = np.ones((1, 32), f32).astype(bf16)
    onesc = np.ones((128, 2), f32).astype(bf16)

    in_maps = []
    for cidx in range(NCORES):
        sl = slice(cidx * BL, (cidx + 1) * BL)
        w0 = (cidx * BL) % NW
        in_maps.append({
            "x": xs[sl], "eb": eb[w0:w0 + BL],
            "wq": wq, "wk": wk, "wv": wv, "wp": wp,
            "onesr": onesr, "onesb": onesb, "onesc": onesc,
        })
    return in_maps


def kernel(x_q, x_kv, mask, q_w, q_b, kv_w, kv_b, proj_w, proj_b,
           rpb_table, rpi):
    nc = _get_program()
    in_maps = _prep_inputs(x_q, x_kv, mask, q_w, q_b, kv_w, kv_b,
                           proj_w, proj_b, rpb_table, rpi)
    res = run_bass_kernel_spmd(nc, in_maps, core_ids=list(range(NCORES)),
                               trace=False)
    out = np.concatenate([res.results[i]["out"] for i in range(NCORES)], 0)
    return np.ascontiguousarray(out.astype(np.float32))


def run_traced(inputs, trace=True):
    """test-harness entry: returns (output, exec_time_ns, results_obj)."""
    nc = _get_program()
    in_maps = _prep_inputs(**inputs)
    res = run_bass_kernel_spmd(nc, in_maps, core_ids=list(range(NCORES)),
                               trace=trace)
    out = np.concatenate([res.results[i]["out"] for i in range(NCORES)], 0)
    return np.ascontiguousarray(out.astype(np.float32)), res.exec_time_ns, res



# revision 15
# speedup vs baseline: 1.5474x; 1.5474x over previous
"""CrossWindowAttention Trainium2 kernel (8 NeuronCores, window-resident v3).

v3 redesign vs v2 (902us):
  - resharded: core c takes b in {64*g + 8c + j} -> only 8 distinct mask
    windows per core; the exp(mask+rpb) bias table (12.7MB bf16) is loaded
    once and stays SBUF-resident. HBM traffic drops 62MB -> ~30MB/core.
  - PV uses e2 as the matmul *stationary* (out [q, c] layout): out free
    size is 33/head instead of 344, and a ones-column in the augmented V
    yields the softmax row-sum s for free as column 32 of each head block.
    Kills v2's s-row matmuls (6192 cyc/batch) and r-broadcast matmuls
    (2064 cyc/batch).
  - normalization r=1/s is per-partition in the [q, c] layout: DVE
    reciprocal + one strided multiply on PSUM evacuation.
  - PE transpose (identity matmul) maps u [q,c] -> ut [c,q] for the out
    projection; ut chunk 1 carries a persistent ones row for proj bias.
  - bias application split across engines: head-groups g0 -> GpSimd mul,
    g1 -> PE identity-fold of additive bias into PSUM (exp(A+B) path),
    g2 -> DVE mul.  ACT does only the 9 exp ops.
"""

import sys

if "/opt/trn_rl_repo" not in sys.path:
    sys.path.insert(0, "/opt/trn_rl_repo")

import numpy as np
import ml_dtypes
from contextlib import ExitStack

import concourse.bass as bass
import concourse.tile as tile
from concourse import bacc, mybir
from concourse.bass_utils import run_bass_kernel_spmd

dt = mybir.dt

B = 256
N = 343          # tokens per window
NP = 344         # padded token dim
C = 192
H = 6
D = 32
NW = 64
NCORES = 8
BL = B // NCORES
NWIN = 8         # distinct windows per core
CA = 194         # c + ones row + zero pad row
SCALE = D ** -0.5

BF16 = dt.bfloat16
F32 = dt.float32

DEBUG = False

KCH = [(0, 128), (128, 66)]              # contraction chunks of augmented c
MCH = [(0, 96), (96, 96)]                # q/k head-tile chunks
TCH = [(0, 128), (128, 128), (256, 88)]  # token chunks (row-padded to 344)
FOLD_G = (1,)                            # head-groups using PE bias-fold
POOL_G = (0,)                            # head-groups whose eb-mul runs on GpSimd


def _patch_act_tables():
    """Force one ACT table set covering Exp (avoids ACT_TABLE_LOAD ~9.5us)."""
    import concourse.bacc as bacc_mod
    if getattr(bacc_mod, "_act_tables_patched", False):
        return
    real = bacc_mod.get_activation_tables

    def patched(arch):
        t = real(arch)
        return {k: (v if k == "natural_log_exp_and_others" else set())
                for k, v in t.items()}

    bacc_mod.get_activation_tables = patched
    bacc_mod._act_tables_patched = True


def blist_for_core(cidx):
    return [64 * g + 8 * cidx + j for g in range(4) for j in range(8)]


def build_program():
    _patch_act_tables()
    nc = bacc.Bacc("TRN2", target_bir_lowering=False, debug=False,
                   num_devices=NCORES)

    d_x = nc.dram_tensor("x", [BL, 2, CA, NP], BF16, kind="ExternalInput").ap()
    d_ebr = nc.dram_tensor("ebr", [NWIN, 128, 3, H, NP], BF16,
                           kind="ExternalInput").ap()
    d_wq = nc.dram_tensor("wq", [CA, C], BF16, kind="ExternalInput").ap()
    d_wk = nc.dram_tensor("wk", [CA, C], BF16, kind="ExternalInput").ap()
    d_wv = nc.dram_tensor("wv", [CA, 198], BF16, kind="ExternalInput").ap()
    d_wpa0 = nc.dram_tensor("wpa0", [96, C], BF16, kind="ExternalInput").ap()
    d_wpa1 = nc.dram_tensor("wpa1", [97, C], BF16, kind="ExternalInput").ap()
    d_id = nc.dram_tensor("ident", [128, 128], BF16, kind="ExternalInput").ap()
    d_ones = nc.dram_tensor("onesr", [1, NP], BF16, kind="ExternalInput").ap()
    d_out = nc.dram_tensor("out", [BL, N, C], F32, kind="ExternalOutput").ap()
    dbg = {}
    if DEBUG:
        for nm, shp in [("dbg_qk0", [96, 2, NP]), ("dbg_v0", [128, 198]),
                        ("dbg_e2_00", [128, 2, NP]), ("dbg_e2_10", [128, 2, NP]),
                        ("dbg_us0", [128, 6, 32]), ("dbg_ut0", [96, NP]),
                        ("dbg_ut1", [97, NP])]:
            dbg[nm] = nc.dram_tensor(nm, shp, BF16, kind="ExternalOutput").ap()
        for nm, shp in [("dbg_u0", [128, 198]), ("dbg_s0", [128, 6]),
                        ("dbg_rs0", [128, 6])]:
            dbg[nm] = nc.dram_tensor(nm, shp, F32, kind="ExternalOutput").ap()

    with tile.TileContext(nc) as tc, ExitStack() as ctx:
        cpool = ctx.enter_context(tc.tile_pool(name="const", bufs=1))
        xpool = ctx.enter_context(tc.tile_pool(name="x", bufs=2))
        qkpool = ctx.enter_context(tc.tile_pool(name="qk", bufs=2))
        vpool = ctx.enter_context(tc.tile_pool(name="v", bufs=2))
        epool = ctx.enter_context(tc.tile_pool(name="e", bufs=3))
        e2pool = ctx.enter_context(tc.tile_pool(name="e2", bufs=3))
        uspool = ctx.enter_context(tc.tile_pool(name="us", bufs=2))
        rspool = ctx.enter_context(tc.tile_pool(name="rs", bufs=2))
        utpool = ctx.enter_context(tc.tile_pool(name="ut", bufs=2))
        fpool = ctx.enter_context(tc.tile_pool(name="fin", bufs=2))
        ps_a = ctx.enter_context(tc.tile_pool(name="ps_a", bufs=2, space="PSUM"))
        ps_b = ctx.enter_context(tc.tile_pool(name="ps_b", bufs=3, space="PSUM"))
        ps_t = ctx.enter_context(tc.tile_pool(name="ps_t", bufs=1, space="PSUM"))

        # ---- resident constants ----
        wq_t, wk_t, wv_t = [], [], []
        for ki, (ko, kn) in enumerate(KCH):
            for lst, src, nm, fc in ((wq_t, d_wq, "wq", C), (wk_t, d_wk, "wk", C),
                                     (wv_t, d_wv, "wv", 198)):
                t = cpool.tile([kn, fc], BF16, tag=f"{nm}{ki}", name=f"{nm}{ki}")
                nc.sync.dma_start(t[:], src[ko:ko + kn, :])
                lst.append(t)
        wpa0 = cpool.tile([96, C], BF16, tag="wpa0")
        nc.sync.dma_start(wpa0[:], d_wpa0[:, :])
        wpa1 = cpool.tile([97, C], BF16, tag="wpa1")
        nc.sync.dma_start(wpa1[:], d_wpa1[:, :])
        ident = cpool.tile([128, 128], BF16, tag="ident")
        nc.sync.dma_start(ident[:], d_id[:, :])
        ebr_t = []
        for wl in range(NWIN):
            t = cpool.tile([128, 3, H, NP], BF16, tag=f"ebr{wl}", name=f"ebr{wl}")
            nc.sync.dma_start(t[:], d_ebr[wl])
            ebr_t.append(t)

        def emit_head(i, tail_fn=None):
            # ---- input DMAs (Sync hwdge) ----
            x_t = []
            for ki, (ko, kn) in enumerate(KCH):
                t = xpool.tile([kn, 2, NP], BF16, tag=f"x{ki}", name=f"x{ki}")
                nc.sync.dma_start(t[:], d_x[i, :, ko:ko + kn, :].rearrange("a k n -> k a n"))
                x_t.append(t)

            # ---- previous batch's tail ----
            if tail_fn is not None:
                tail_fn()

            # ---- Q^T / K^T projections (planes: 0=q, 1=k) ----
            qk_sb = []
            for mi, (mo, mn) in enumerate(MCH):
                ps = ps_a.tile([128, 2, 512], F32, tag="a", name="mmqk")
                for pl, w_t, xi in ((0, wq_t, 0), (1, wk_t, 1)):
                    for ki, (ko, kn) in enumerate(KCH):
                        nc.tensor.matmul(ps[0:mn, pl, 0:NP],
                                         w_t[ki][:, mo:mo + mn],
                                         x_t[ki][:, xi, :],
                                         start=(ki == 0), stop=(ki == len(KCH) - 1))
                sb = qkpool.tile([96, 2, NP], BF16, tag=f"qk{mi}", name=f"qk{mi}")
                nc.vector.tensor_copy(sb[:], ps[0:mn, 0:2, 0:NP])
                if DEBUG and i == 0 and mi == 0:
                    nc.sync.dma_start(dbg["dbg_qk0"][:, :, :], sb[:])
                qk_sb.append(sb)

            # ---- V projection (augmented: ones col per head at 33h+32) ----
            v_t = []
            for ti, (to, tn) in enumerate(TCH):
                ps = ps_a.tile([128, 2, 512], F32, tag="a", name="mmv")
                for ki, (ko, kn) in enumerate(KCH):
                    nc.tensor.matmul(ps[0:tn, 0, 0:198],
                                     x_t[ki][:, 1, to:to + tn],
                                     wv_t[ki][:],
                                     start=(ki == 0), stop=(ki == len(KCH) - 1))
                sb = vpool.tile([128, 198], BF16, tag=f"v{ti}", name=f"v{ti}")
                nc.vector.tensor_copy(sb[0:tn, :], ps[0:tn, 0, 0:198])
                if DEBUG and i == 0 and ti == 0:
                    nc.sync.dma_start(dbg["dbg_v0"][:, :], sb[:])
                v_t.append(sb)
            return qk_sb, v_t

        def head_tile(h):
            return h // 3, 32 * (h % 3)

        def emit_attn(i, wl, qk_sb, v_t):
            # u accumulators, one per q-chunk, [q, 6*33] layout in a bank
            u_ps = [ps_b.tile([128, 512], F32, tag="b", name=f"u{qi}")
                    for qi in range(3)]
            eb = ebr_t[wl]
            for g in range(3):
                for ci, (co, cn) in enumerate(TCH):
                    a_ps = ps_a.tile([128, 2, 512], F32, tag="a", name="aqk")
                    fold = g in FOLD_G
                    for hh in range(2):
                        h = 2 * g + hh
                        t_i, r_off = head_tile(h)
                        nc.tensor.matmul(
                            a_ps[0:cn, hh, 0:NP],
                            qk_sb[t_i][r_off:r_off + D, 1, co:co + cn],
                            qk_sb[t_i][r_off:r_off + D, 0, :],
                            start=True, stop=not fold,
                            skip_group_check=True)
                    if fold:
                        for hh in range(2):
                            nc.tensor.matmul(
                                a_ps[0:cn, hh, 0:NP],
                                ident[0:cn, 0:cn],
                                eb[0:cn, ci, 2 * g + hh, :],
                                start=False, stop=True,
                                skip_group_check=True)
                        e2 = e2pool.tile([128, 2, NP], BF16, tag="e2", name="e2")
                        nc.scalar.activation(e2[0:cn, :, :], a_ps[0:cn, 0:2, 0:NP],
                                             mybir.ActivationFunctionType.Exp)
                    else:
                        e_t = epool.tile([128, 2, NP], BF16, tag="e", name="e")
                        nc.scalar.activation(e_t[0:cn, :, :], a_ps[0:cn, 0:2, 0:NP],
                                             mybir.ActivationFunctionType.Exp)
                        e2 = e2pool.tile([128, 2, NP], BF16, tag="e2", name="e2")
                        eng = nc.gpsimd if g in POOL_G else nc.vector
                        eng.tensor_mul(e2[0:cn, :, :], e_t[0:cn, :, :],
                                       eb[0:cn, ci, 2 * g:2 * g + 2, :])
                    if DEBUG and i == 0 and ci == 0 and g in (0, 1):
                        nc.sync.dma_start(dbg[f"dbg_e2_{g}0"][:, :, :], e2[:])
                    # PV: e2 stationary, v_aug moving; accumulate over ci
                    # NB: start=True clears has_written for the WHOLE bank, so
                    # only the very first matmul into each u bank may set it;
                    # later first-writes to fresh columns overwrite (bit clear).
                    for hh in range(2):
                        h = 2 * g + hh
                        for qi, (qo, qn) in enumerate(TCH):
                            nc.tensor.matmul(
                                u_ps[qi][0:qn, 33 * h:33 * h + 33],
                                e2[0:cn, hh, qo:qo + qn],
                                v_t[ci][0:cn, 33 * h:33 * h + 33],
                                start=(g == 0 and ci == 0 and hh == 0),
                                stop=(g == 2 and ci == len(TCH) - 1 and hh == 1),
                                skip_group_check=True)
            return u_ps

        def emit_tail(i, u_ps):
            # normalize: r = 1/s per (q, head); evac u -> bf16
            us_sb = []
            ut_ps = ps_t.tile([128, 2, NP], BF16, tag="t", name="ut_ps")
            for qi, (qo, qn) in enumerate(TCH):
                u6 = u_ps[qi][0:qn, 0:198].rearrange("p (h x) -> p h x", x=33)
                s_sb = rspool.tile([128, 6], F32, tag=f"s{qi}", name=f"s{qi}")
                nc.vector.tensor_copy(s_sb[0:qn, :], u6[:, :, 32])
                rs = rspool.tile([128, 6], F32, tag=f"rs{qi}", name=f"rs{qi}")
                nc.vector.reciprocal(rs[0:qn, :], s_sb[0:qn, :])
                if DEBUG and i == 0 and qi == 0:
                    ucp = uspool.tile([128, 198], F32, tag="ucp", name="ucp")
                    nc.vector.tensor_copy(ucp[0:qn, :], u_ps[qi][0:qn, 0:198])
                    nc.sync.dma_start(dbg["dbg_u0"][:, :], ucp[:])
                    nc.sync.dma_start(dbg["dbg_s0"][:, :], s_sb[:])
                    nc.sync.dma_start(dbg["dbg_rs0"][:, :], rs[:])
                us = uspool.tile([128, 6, 32], BF16, tag=f"us{qi}", name=f"us{qi}")
                nc.vector.tensor_mul(us[0:qn, :, :], u6[:, :, 0:32],
                                     rs[0:qn, :, None].broadcast_to([qn, 6, 32]))
                us_sb.append(us)
                if DEBUG and i == 0 and qi == 0:
                    nc.sync.dma_start(dbg["dbg_us0"][:, :, :], us[:])
                # transpose chunks into the ut bank as soon as each us is ready
                for ch in range(2):
                    nc.tensor.transpose(ut_ps[0:96, ch, qo:qo + qn],
                                        us[0:qn, 3 * ch:3 * ch + 3, 0:32],
                                        ident[0:qn, 0:qn])
            # evac transposed chunks; ut1 carries persistent ones row (96)
            ut0 = utpool.tile([96, NP], BF16, tag="ut0", name="ut0")
            nc.vector.tensor_copy(ut0[:], ut_ps[0:96, 0, 0:NP])
            ut1 = utpool.tile([97, NP], BF16, tag="ut1", name="ut1")
            nc.vector.tensor_copy(ut1[0:96, :], ut_ps[0:96, 1, 0:NP])
            if i < 2:  # bufs=2: the ones row persists per slot
                nc.sync.dma_start(ut1[96:97, :], d_ones[:, :])
            if DEBUG and i == 0:
                nc.sync.dma_start(dbg["dbg_ut0"][:, :], ut0[:])
                nc.sync.dma_start(dbg["dbg_ut1"][0:96, :], ut1[0:96, :])
            # out projection + store
            for qi, (qo, qn) in enumerate(TCH):
                o_ps = ps_b.tile([128, 512], F32, tag="b", name=f"o{qi}")
                nc.tensor.matmul(o_ps[0:qn, 0:C], ut0[:, qo:qo + qn], wpa0[:],
                                 start=True, stop=False)
                nc.tensor.matmul(o_ps[0:qn, 0:C], ut1[:, qo:qo + qn], wpa1[:],
                                 start=False, stop=True)
                f_sb = fpool.tile([128, C], F32, tag=f"f{qi}", name=f"f{qi}")
                nc.vector.tensor_copy(f_sb[0:qn, :], o_ps[0:qn, 0:C])
                rows = min(qn, N - qo)
                nc.sync.dma_start(d_out[i, qo:qo + rows, :], f_sb[0:rows, :])

        # software pipeline: batch i's tail runs inside batch i+1's head
        prev = None
        for i in range(BL):
            wl = i % NWIN
            if prev is not None:
                pi, pu = prev
                tail_fn = lambda pi=pi, pu=pu: emit_tail(pi, pu)
            else:
                tail_fn = None
            qk_sb, v_t = emit_head(i, tail_fn)
            u_ps = emit_attn(i, wl, qk_sb, v_t)
            prev = (i, u_ps)
        emit_tail(prev[0], prev[1])

    nc.compile()
    return nc


_NC_CACHE = None


def _get_program():
    global _NC_CACHE
    if _NC_CACHE is None:
        _NC_CACHE = build_program()
    return _NC_CACHE


def _prep_inputs(x_q, x_kv, mask, q_w, q_b, kv_w, kv_b, proj_w, proj_b,
                 rpb_table, rpi):
    bf16 = ml_dtypes.bfloat16
    f32 = np.float32

    def aug_w(w, bias, scale=1.0):
        m = np.zeros((CA, C), f32)
        m[:C] = np.asarray(w, f32).T
        m[C] = np.asarray(bias, f32)
        return m * scale

    wq = aug_w(q_w, q_b, SCALE).astype(bf16)
    wk = aug_w(kv_w[:C], kv_b[:C]).astype(bf16)
    wv_base = aug_w(kv_w[C:], kv_b[C:])
    wva = np.zeros((CA, 198), f32)
    for h in range(H):
        wva[:, 33 * h:33 * h + 32] = wv_base[:, 32 * h:32 * h + 32]
        wva[C, 33 * h + 32] = 1.0
    wva = wva.astype(bf16)
    wpT = np.asarray(proj_w, f32).T
    wpa0 = np.ascontiguousarray(wpT[0:96]).astype(bf16)
    wpa1 = np.concatenate([wpT[96:192], np.asarray(proj_b, f32)[None]],
                          0).astype(bf16)
    ident = np.eye(128, dtype=f32).astype(bf16)
    onesr = np.ones((1, NP), f32).astype(bf16)

    def xT_aug(x):
        out = np.zeros((B, CA, NP), f32)
        out[:, :C, :N] = np.asarray(x, f32).transpose(0, 2, 1)
        out[:, C, :N] = 1.0
        return out

    xs = np.stack([xT_aug(x_q), xT_aug(x_kv)], 1).astype(bf16)  # [B,2,CA,NP]

    # bias table per window, packed [128, 3, H, NP]:
    # heads in FOLD groups stay additive (B); others hold exp(B)
    g = np.asarray(rpb_table, f32)[np.asarray(rpi)]        # [q, k, H]
    rpbT = np.zeros((NP, H, NP), f32)
    rpbT[:N, :, :N] = g.transpose(1, 2, 0)                 # [k, h, q]
    fold_heads = {2 * gg + hh for gg in FOLD_G for hh in range(2)}
    ebr = np.empty((NW, 128, 3, H, NP), bf16)
    maskT = np.full((NP, NP), -100.0, f32)
    for w in range(NW):
        maskT[:N, :N] = np.asarray(mask[w], f32).T
        Bm = maskT[:, None, :] + rpbT                      # [k, h, q]
        Bm[:, :, N] = -100.0
        Bm[0, :, N] = 0.0
        for h in range(H):
            if h not in fold_heads:
                Bm[:, h] = np.exp(Bm[:, h])
        pad = np.zeros((384, H, NP), f32)
        pad[:NP] = Bm
        ebr[w] = pad.reshape(3, 128, H, NP).transpose(1, 0, 2, 3).astype(bf16)

    in_maps = []
    for cidx in range(NCORES):
        bl = blist_for_core(cidx)
        in_maps.append({
            "x": np.ascontiguousarray(xs[bl]),
            "ebr": np.ascontiguousarray(ebr[8 * cidx:8 * cidx + 8]),
            "wq": wq, "wk": wk, "wv": wva,
            "wpa0": wpa0, "wpa1": wpa1,
            "ident": ident, "onesr": onesr,
        })
    return in_maps


def _gather(res):
    out = np.empty((B, N, C), np.float32)
    for cidx in range(NCORES):
        out[blist_for_core(cidx)] = res.results[cidx]["out"]
    return out


def kernel(x_q, x_kv, mask, q_w, q_b, kv_w, kv_b, proj_w, proj_b,
           rpb_table, rpi):
    nc = _get_program()
    in_maps = _prep_inputs(x_q, x_kv, mask, q_w, q_b, kv_w, kv_b,
                           proj_w, proj_b, rpb_table, rpi)
    res = run_bass_kernel_spmd(nc, in_maps, core_ids=list(range(NCORES)),
                               trace=False)
    return np.ascontiguousarray(_gather(res))


def run_traced(inputs, trace=True):
    """test-harness entry: returns (output, exec_time_ns, results_obj)."""
    nc = _get_program()
    in_maps = _prep_inputs(**inputs)
    res = run_bass_kernel_spmd(nc, in_maps, core_ids=list(range(NCORES)),
                               trace=trace)
    return np.ascontiguousarray(_gather(res)), res.exec_time_ns, res
